# revision 22
# baseline (speedup 1.0000x reference)
"""Trainium2 Bass kernel for nn_BuildVolume2dChaos (bilinear-warp cost volume).

kernel(refimg_fea, targetimg_fea, disps) -> volume [B=2, D=32, H=128, W=256]

Self-contained: builds an SPMD Bass program (one per-core variant), shards
inputs over 8 NeuronCores as (b, h-slice) = (core//4, 32*(core%4)), runs via
concourse.bass_utils.run_bass_kernel_spmd, reassembles the full output.

Algorithm per core (b fixed, 32 h-rows), per h-row g:
  the horizontal bilinear warp is a banded matmul  warped[c,(d,w)] =
  sum_{w''} Tv[c,w''] * relu(1 - |ix(d,w) - w''|)  with ix =
  (w - disp)*W/(W-1) - 0.5 and zero-padded, vertically-lerped target rows Tv.
  W splits into tiles [62,62,62,62,8] so each tile's 128-row w''-window fits
  the PE contraction. Per row: the tile-offset ix row broadcasts to 128
  partitions with one stride-0 DRAM->SBUF DMA, two in-place activations form
  the tent weights for all tiles at once, 17 N<=512 matmuls (4 w-tiles packed
  into the 128 PSUM partitions via tile_position) produce warped, |ref -
  warped| reduces over channels with one-hot matmuls accumulating 8 rows per
  PSUM bank group.
"""
import sys

sys.path.insert(0, '/opt/trn_rl_repo')

import numpy as np
import bass_rust
import concourse.bass as bass
import concourse.mybir as mybir
from concourse.tile import TileContext
from concourse.vector_clock import ScopedClock

f32 = mybir.dt.float32
Alu = mybir.AluOpType
ActF = mybir.ActivationFunctionType

B, C, H, W, D = 2, 32, 128, 256, 32
HS = 32
NCORES = 8
XSCALE = W / (W - 1)

TBASE = [0, 62, 124, 186, 248]
TSIZE = [62, 62, 62, 62, 8]
TOFF = [b - 65 for b in TBASE]          # w''-window start per tile
NMAIN = 4 * 32 * 62                      # 7936 main-tile columns (k<4)
NK4 = 32 * 8                             # 256 k4 columns
NALL = NMAIN + NK4                       # 8192

_MAX_WAITS = 1


def _split_excess_waits(nc, max_waits=_MAX_WAITS):
    """Walrus (this neuronx-cc XLA path) rejects instructions carrying more
    than ~1 sem-wait ('Too many sync wait commands'). Hoist excess waits onto
    same-engine Drain instructions inserted immediately before."""
    n_fixed = 0
    for f in nc.m.functions:
        for bb in f.blocks:
            insts = bb.instructions
            i = 0
            while i < len(insts):
                ins = insts[i]
                si = ins.sync_info
                if si is not None and si.on_wait and len(si.on_wait) > max_waits:
                    waits = list(si.on_wait)
                    ins.sync_info = bass_rust.SyncInfo(
                        on_wait=waits[:max_waits], on_update=list(si.on_update))
                    pre = []
                    for jj in range(max_waits, len(waits), max_waits):
                        d = mybir.InstDrain(
                            name=f"{ins.name}-ws{jj}", ins=[], outs=[])
                        d.engine = ins.engine
                        d.sync_info = bass_rust.SyncInfo(
                            on_wait=waits[jj:jj + max_waits], on_update=[])
                        pre.append(d)
                    for d in reversed(pre):
                        insts.insert(i, d)
                        nc.register_instruction(d, overwrite=True)
                    i += len(pre)
                    n_fixed += 1
                i += 1
    return n_fixed


class _PatchedTileContext(TileContext):
    """Walrus CoreV3 rejects instructions with >1 sem-wait ('Too many sync
    wait commands'); split the kernel-tail drain's waits across drains."""

    def __exit__(self, exc_type, exc_val, exc_tb):
        ret = super().__exit__(exc_type, exc_val, exc_tb)
        if exc_type is None:
            _split_excess_waits(self.nc)
        return ret

    def _drain_and_barrier(self, tick_clock, wait_clock):
        nc = self.nc
        drain_inst = nc.sync.drain()
        wait_clock.add_sem_waits(
            drain_inst.ins, ScopedClock({None: tick_clock.global_clock})
        )
        si = drain_inst.ins.sync_info
        if si is not None and si.on_wait and len(si.on_wait) > _MAX_WAITS:
            waits = list(si.on_wait)
            drain_inst.ins.sync_info = bass_rust.SyncInfo(
                on_wait=waits[:_MAX_WAITS], on_update=list(si.on_update)
            )
            for i in range(_MAX_WAITS, len(waits), _MAX_WAITS):
                extra = nc.sync.drain()
                extra.ins.sync_info = bass_rust.SyncInfo(
                    on_wait=waits[i: i + _MAX_WAITS], on_update=[]
                )
        nc.all_engine_barrier()
        assert self.sems is not None
        popped = nc._tile_sem_poison_stack.pop()
        assert popped is self._sem_poison
        nc.clear_and_free_semaphores(list(self.sems.allocated().values()))
        nc.all_engine_barrier()


def build_nc(reps=1):
    nc = bass.Bass("TRN2", debug=False, enable_asserts=False)

    dispst = nc.dram_tensor("dispst", [HS, NALL], f32, kind="ExternalInput")
    wrowc = nc.dram_tensor("wrowc", [HS, NALL], f32, kind="ExternalInput")
    tga = [nc.dram_tensor(f"tga{k}", [128, HS, C], f32, kind="ExternalInput")
           for k in range(5)]
    tgb = [nc.dram_tensor(f"tgb{k}", [128, HS, C], f32, kind="ExternalInput")
           for k in range(5)]
    wyb = nc.dram_tensor("wyb", [128, HS], f32, kind="ExternalInput")
    refst = nc.dram_tensor("refst", [128, HS * 62], f32, kind="ExternalInput")
    ref4st = nc.dram_tensor("ref4st", [C, HS * 8], f32, kind="ExternalInput")
    redsel = nc.dram_tensor("redsel", [128, 8 * 32], f32, kind="ExternalInput")
    redsel4 = nc.dram_tensor("redsel4", [C, 8 * 32], f32, kind="ExternalInput")
    pcol = nc.dram_tensor("pcol", [128, 1], f32, kind="ExternalInput")
    ix_dram = nc.dram_tensor("ix_dram", [HS, NALL], f32, kind="Internal")
    # packed outputs: host unscrambles (gq, gp*4+k, d, w62) -> [D, HS, W]
    vol = nc.dram_tensor("vol", [4, 32, 1984], f32, kind="ExternalOutput")
    vol4 = nc.dram_tensor("vol4", [4, 8, 256], f32, kind="ExternalOutput")

    with _PatchedTileContext(nc) as tc:
        with tc.tile_pool(name="const", bufs=1) as cpool:
            s_tva = [cpool.tile([128, HS, C], f32, tag=f"tva{k}",
                                name=f"tva{k}") for k in range(5)]
            s_ref = cpool.tile([128, HS * 62], f32, tag="ref")
            nc.sync.dma_start(s_ref[:, :], refst[:, :])
            s_ref4 = cpool.tile([C, HS * 8], f32, tag="ref4")
            nc.sync.dma_start(s_ref4[:, :], ref4st[:, :])
            s_red = cpool.tile([128, 8, 32], f32, tag="red")
            nc.sync.dma_start(
                s_red[:, :, :],
                redsel[:, :].rearrange("p (g m) -> p g m", g=8))
            s_red4 = cpool.tile([C, 8, 32], f32, tag="red4")
            nc.sync.dma_start(
                s_red4[:, :, :],
                redsel4[:, :].rearrange("p (g m) -> p g m", g=8))
            s_pc = cpool.tile([128, 1], f32, tag="pc")
            nc.sync.dma_start(s_pc[:, :], pcol[:, :])

            # ---- prologue: ix affine map, then park it in DRAM ----
            with tc.tile_pool(name="ixp", bufs=1) as xpool:
                s_disp = xpool.tile([HS, NALL], f32, tag="disp")
                nc.sync.dma_start(s_disp[:, :], dispst[:, :])
                s_wr = xpool.tile([HS, NALL], f32, tag="wr")
                nc.sync.dma_start(s_wr[:, :], wrowc[:, :])
                s_ix = xpool.tile([HS, NALL], f32, tag="ix")
                nc.vector.scalar_tensor_tensor(
                    s_ix[:, :], s_disp[:, :], -XSCALE, s_wr[:, :],
                    Alu.mult, Alu.add)
                nc.sync.dma_start(ix_dram[:, :], s_ix[:, :])

            # ---- prologue: vertical lerp of target rows ----
            with tc.tile_pool(name="lerp", bufs=1) as lpool:
                s_wyb = lpool.tile([128, HS], f32, tag="wyb")
                nc.sync.dma_start(s_wyb[:, :], wyb[:, :])
                wyb_b = s_wyb[:, :].unsqueeze(2).broadcast_to([128, HS, C])
                for k in range(5):
                    ta = lpool.tile([128, HS, C], f32, tag="ta")
                    tb = lpool.tile([128, HS, C], f32, tag="tb")
                    nc.sync.dma_start(ta[:, :, :], tga[k][:, :, :])
                    nc.sync.dma_start(tb[:, :, :], tgb[k][:, :, :])
                    u = lpool.tile([128, HS, C], f32, tag="u")
                    nc.vector.tensor_tensor(u[:, :, :], tb[:, :, :],
                                            ta[:, :, :], Alu.subtract)
                    nc.vector.tensor_tensor(u[:, :, :], u[:, :, :], wyb_b,
                                            Alu.mult)
                    nc.vector.tensor_tensor(s_tva[k][:, :, :], ta[:, :, :],
                                            u[:, :, :], Alu.add)

            with (
                tc.tile_pool(name="work", bufs=1) as wpool,
                tc.tile_pool(name="outs", bufs=1) as opool,
                tc.tile_pool(name="pw", bufs=1, space="PSUM") as pw_pool,
                tc.tile_pool(name="pw4", bufs=1, space="PSUM") as pw4_pool,
                tc.tile_pool(name="pr", bufs=1, space="PSUM") as pr_pool,
                tc.tile_pool(name="pr4", bufs=1, space="PSUM") as pr4_pool,
            ):
                s_adf = wpool.tile([128, NALL // 4 + NK4], f32, tag="adf")
                for rep in range(reps):
                    for gq in range(4):
                        rzd = [pr_pool.tile([32, 496], f32, tag=f"rz{dq}",
                                            name=f"rz{dq}")
                               for dq in range(4)]
                        rz4 = pr4_pool.tile([32, 256], f32, tag="rz4")
                        for gp in range(8):
                            g = gq * 8 + gp
                            s_yvt = wpool.tile([128, NALL], f32, tag="yv")
                            s_yv = s_yvt[:, :]
                            nc.sync.dma_start(
                                s_yv,
                                ix_dram[g:g + 1, :].broadcast_to([128, NALL]))
                            nc.scalar.activation(
                                s_yv, s_yv, ActF.Abs,
                                bias=s_pc[:, 0:1], scale=1.0)
                            nc.scalar.activation(
                                s_yv, s_yv, ActF.Relu,
                                bias=1.0, scale=-1.0)
                            ref_b = s_ref[:, g * 62:(g + 1) * 62].unsqueeze(
                                1).unsqueeze(1).broadcast_to([128, 2, 8, 62])
                            for half in range(2):
                                # two 496-col chunks in one bank-aligned tile
                                wbig = pw_pool.tile([128, 1024], f32,
                                                    tag="wbig")
                                for j in range(2):
                                    dq = half * 2 + j
                                    for k in range(4):
                                        if k == 0:
                                            # w'' < 0 rows of tile 0 are zero
                                            nc.tensor.matmul(
                                                wbig[0:32, j * 512:
                                                     j * 512 + 496],
                                                s_tva[0][64:128, g, :],
                                                s_yv[64:128, dq * 496:
                                                     (dq + 1) * 496],
                                                start=True, stop=True,
                                                tile_position=(64, 0))
                                        else:
                                            nc.tensor.matmul(
                                                wbig[32 * k:32 * (k + 1),
                                                     j * 512:j * 512 + 496],
                                                s_tva[k][:, g, :],
                                                s_yv[:, k * 1984 + dq * 496:
                                                     k * 1984 +
                                                     (dq + 1) * 496],
                                                start=True, stop=True,
                                                tile_position=(0, 32 * k))
                                # df = ref - warped straight from PSUM
                                wview = wbig[:, :].rearrange(
                                    "p (q b) -> p q b", q=2)[:, :, 0:496]
                                nc.vector.tensor_tensor(
                                    s_adf[:, half * 992:(half + 1) * 992]
                                    .rearrange("p (q d w) -> p q d w",
                                               q=2, d=8),
                                    ref_b,
                                    wview.rearrange("p q (d w) -> p q d w",
                                                    d=8),
                                    Alu.subtract)
                            w4 = pw4_pool.tile([32, 256], f32, tag="w4")
                            nc.tensor.matmul(
                                w4[:, :], s_tva[4][0:74, g, :],
                                s_yv[0:74, NMAIN:NALL],
                                start=True, stop=True, tile_position=(0, 0))
                            ref4_b = s_ref4[:, g * 8:(g + 1) * 8].unsqueeze(
                                1).broadcast_to([C, 32, 8])
                            nc.vector.tensor_tensor(
                                s_adf[0:32, 1984:2240].rearrange(
                                    "p (d w) -> p d w", d=32),
                                ref4_b,
                                w4[:, :].rearrange(
                                    "p (d w) -> p d w", d=32),
                                Alu.subtract)
                            # adf = |df| over everything at once (in place)
                            nc.vector.scalar_tensor_tensor(
                                s_adf[:, :], s_adf[:, :], -1.0,
                                s_adf[:, :], Alu.mult, Alu.max)
                            for dq in range(4):
                                nc.tensor.matmul(
                                    rzd[dq][:, :],
                                    s_red[:, gp, :],
                                    s_adf[:, dq * 496:(dq + 1) * 496],
                                    start=(gp == 0), stop=(gp == 7))
                            nc.tensor.matmul(
                                rz4[:, :], s_red4[:, gp, :],
                                s_adf[0:32, 1984:2240],
                                start=(gp == 0), stop=(gp == 7))
                        s_volq = opool.tile([32, 1984], f32, tag="volq")
                        for dq in range(4):
                            nc.vector.tensor_copy(
                                s_volq[:, dq * 496:(dq + 1) * 496],
                                rzd[dq][:, :])
                        nc.sync.dma_start(vol[gq, :, :], s_volq[:, :])
                        s_vol4q = opool.tile([8, 256], f32, tag="vol4q")
                        nc.vector.tensor_copy(s_vol4q[:, :], rz4[0:8, :])
                        nc.sync.dma_start(vol4[gq, :, :], s_vol4q[:, :])
    return nc


def _vertical_rows():
    h = np.arange(H)
    iy = h * (H / (H - 1)) - 0.5
    y0 = np.floor(iy).astype(int)
    wy1 = (iy - y0).astype(np.float32)
    return y0, wy1


# (k, dq, d8, w62) | (k4: d32, w8) column layouts, precomputed once
def _col_maps():
    # main: index -> (k, d, w_abs)
    ks = np.repeat(np.arange(4), 32 * 62)
    ds = np.tile(np.repeat(np.arange(32), 62), 4)
    ws = np.tile(np.arange(62), 4 * 32) + TBASE[0] + 62 * ks
    # k4
    d4 = np.repeat(np.arange(32), 8)
    w4 = np.tile(np.arange(8), 32) + 248
    return ks, ds, ws, d4, w4


_KS, _DS, _WS, _D4, _W4 = _col_maps()


def prep_core_inputs(refimg_fea, targetimg_fea, disps, core):
    b = core // 4
    h0 = HS * (core % 4)
    y0, wy1 = _vertical_rows()
    out = {}

    dsl = disps[b, :, h0:h0 + HS, :]                     # [D, HS, W]
    dispst = np.empty((HS, NALL), np.float32)
    dispst[:, :NMAIN] = dsl[_DS, :, _WS].T
    dispst[:, NMAIN:] = dsl[_D4, :, _W4].T
    out["dispst"] = dispst

    wr = np.empty(NALL, np.float32)
    wr[:NMAIN] = _WS * XSCALE - 0.5 - (TOFF[0] + 62 * _KS)
    wr[NMAIN:] = _W4 * XSCALE - 0.5 - TOFF[4]
    out["wrowc"] = np.broadcast_to(wr, (HS, NALL)).copy()

    tgt_t = np.ascontiguousarray(
        targetimg_fea[b].transpose(2, 1, 0))             # [W, H, C]
    gh = h0 + np.arange(HS)
    ra, rb = y0[gh], y0[gh] + 1
    rava = (ra >= 0) & (ra < H)
    rbva = (rb >= 0) & (rb < H)
    for k in range(5):
        wp = TOFF[k] + np.arange(128)
        wvalid = (wp >= 0) & (wp < W)
        ga = np.zeros((128, HS, C), np.float32)
        gb = np.zeros((128, HS, C), np.float32)
        ga[np.ix_(wvalid, rava)] = tgt_t[wp[wvalid]][:, ra[rava], :]
        gb[np.ix_(wvalid, rbva)] = tgt_t[wp[wvalid]][:, rb[rbva], :]
        out[f"tga{k}"] = ga
        out[f"tgb{k}"] = gb
    out["wyb"] = np.broadcast_to(wy1[gh], (128, HS)).copy()

    ref = refimg_fea[b, :, h0:h0 + HS, :]                # [C, HS, W]
    refst = np.empty((128, HS, 62), np.float32)
    for k in range(4):
        refst[32 * k:32 * (k + 1)] = ref[:, :, TBASE[k]:TBASE[k] + 62]
    out["refst"] = refst.reshape(128, HS * 62)
    out["ref4st"] = np.ascontiguousarray(
        ref[:, :, 248:256]).reshape(C, HS * 8)

    redsel = np.zeros((128, 8, 32), np.float32)
    p = np.arange(128)
    for gp in range(8):
        redsel[p, gp, gp * 4 + p // 32] = 1.0
    out["redsel"] = redsel.reshape(128, 8 * 32)
    redsel4 = np.zeros((C, 8, 32), np.float32)
    for gp in range(8):
        redsel4[:, gp, gp] = 1.0
    out["redsel4"] = redsel4.reshape(C, 8 * 32)

    out["pcol"] = -np.arange(128, dtype=np.float32).reshape(128, 1)
    return out


_NC_CACHE = {}


def _get_nc(reps=1):
    if reps not in _NC_CACHE:
        _NC_CACHE[reps] = build_nc(reps=reps)
    return _NC_CACHE[reps]


def run(refimg_fea, targetimg_fea, disps, reps=1):
    from concourse.bass_utils import run_bass_kernel_spmd
    nc = _get_nc(reps=reps)
    in_maps = [prep_core_inputs(refimg_fea, targetimg_fea, disps, core)
               for core in range(NCORES)]
    res = run_bass_kernel_spmd(nc, in_maps, core_ids=list(range(NCORES)))
    full = np.empty((B, D, H, W), np.float32)
    for core in range(NCORES):
        b = core // 4
        h0 = HS * (core % 4)
        # vol: [gq, gp*4+k, (d, w62)] -> [d, 8*gq+gp, 62*k+w62]
        v = res.results[core]["vol"].reshape(4, 8, 4, 32, 62)
        full[b, :, h0:h0 + HS, :248] = v.transpose(3, 0, 1, 2, 4).reshape(
            D, HS, 248)
        v4 = res.results[core]["vol4"].reshape(4, 8, 32, 8)
        full[b, :, h0:h0 + HS, 248:] = v4.transpose(2, 0, 1, 3).reshape(
            D, HS, 8)
    return full


def kernel(refimg_fea, targetimg_fea, disps):
    refimg_fea = np.asarray(refimg_fea, dtype=np.float32)
    targetimg_fea = np.asarray(targetimg_fea, dtype=np.float32)
    disps = np.asarray(disps, dtype=np.float32)
    return run(refimg_fea, targetimg_fea, disps)
